# revision 2
# baseline (speedup 1.0000x reference)
"""CLIP contrastive loss on 8 Trainium2 NeuronCores (Bass/Tile).

Problem: N=16384 paired image/text features, D=512.
  logits = logit_scale * I @ T.T          (16384 x 16384, never materialized)
  loss = (mean_i[lse(row_i) - diag_i] + mean_i[lse(col_i) - diag_i]) / 2

Sharding: data-parallel over N. Core c owns rows [2048c, 2048(c+1)).
Each core computes its 2048-row band of logits_per_image (lhsT = local
I^T columns, rhs = full T^T) and its 2048-row band of logits_per_text
(lhsT = local T^T columns, rhs = full I^T), doing a streaming (online)
logsumexp along the free axis in chunks of 2048 columns:
  - TensorE: 16 bf16 matmuls per (m-tile, chunk), K=512 contracted as
    4x128, accumulated in PSUM [128, 2048] (4 banks, double-buffered).
  - VectorE: negated row-max of the chunk + running-max merge.
  - ScalarE: exp(x - newmax) with per-partition bias AP and fused
    accumulate (row sum) in a single ACTIVATE; plus the rescale factor
    exp(oldmax - newmax).
  - diag: elementwise mul + fused reduce (tensor_tensor_reduce) on the
    natural-layout local features.
Per-core outputs: lse_i, lse_t, diag as [128, 16] (partition p, m-tile
t) = local row t*128+p. The final scalar mean reduction is done on host
in float64 (24 KB of data, vs ~0.5 GFLOP/byte on device).

Inputs are pre-transposed and pre-scaled on host (I is multiplied by
logit_scale before the bf16 cast), so the device program is free of
transposes and of the logit_scale input.
"""

import numpy as np
import ml_dtypes

import concourse.bass as bass
import concourse.tile as tile
from concourse import bacc, mybir
from concourse import bass2jax

# ---- problem geometry (hardcoded per contract) ----
P = 128          # partitions
D = 512          # feature dim
KO = D // P      # 4 contraction sub-tiles
N = 16384        # total rows
C = 8            # cores
NL = N // C      # 2048 local rows per core
MT = NL // P     # 16 m-tiles per core
CH = 2048        # online-softmax chunk (4 PSUM banks)
NCH = N // CH    # 8 chunks
FREE = 512       # matmul moving free dim (1 PSUM bank of fp32)
NSUB = CH // FREE  # 4

F32 = mybir.dt.float32
BF16 = mybir.dt.bfloat16
AX = mybir.AxisListType.X
OP = mybir.AluOpType
ACT = mybir.ActivationFunctionType


def _build_program():
    nc = bacc.Bacc("TRN2", target_bir_lowering=False, debug=False, num_devices=C)

    itf = nc.dram_tensor("itf", [D, N], BF16, kind="ExternalInput").ap()
    ttf = nc.dram_tensor("ttf", [D, N], BF16, kind="ExternalInput").ap()
    itl = nc.dram_tensor("itl", [D, NL], BF16, kind="ExternalInput").ap()
    ttl = nc.dram_tensor("ttl", [D, NL], BF16, kind="ExternalInput").ap()
    iln = nc.dram_tensor("iln", [NL, D], F32, kind="ExternalInput").ap()
    tln = nc.dram_tensor("tln", [NL, D], F32, kind="ExternalInput").ap()
    out_lse = [
        nc.dram_tensor("lse_i", [P, MT], F32, kind="ExternalOutput").ap(),
        nc.dram_tensor("lse_t", [P, MT], F32, kind="ExternalOutput").ap(),
    ]
    out_dg = nc.dram_tensor("dg", [P, MT], F32, kind="ExternalOutput").ap()

    with tile.TileContext(nc) as tc:
        with (
            tc.tile_pool(name="lhs", bufs=2) as lhs_pool,
            tc.tile_pool(name="rhs", bufs=3) as rhs_pool,
            tc.tile_pool(name="psum", bufs=2, space="PSUM") as psum_pool,
            tc.tile_pool(name="exp", bufs=2) as e_pool,
            tc.tile_pool(name="state", bufs=1) as st_pool,
            tc.tile_pool(name="small", bufs=6) as sm_pool,
            tc.tile_pool(name="diag", bufs=2) as d_pool,
        ):
            # ---- diagonal: dg[p, t] = dot(I_scaled[r], T[r]), r = t*128+p
            iln3 = iln.rearrange("(t p) d -> p t d", p=P)
            tln3 = tln.rearrange("(t p) d -> p t d", p=P)
            dgt = st_pool.tile([P, MT], F32, tag="dgt")
            for t in range(MT):
                il = d_pool.tile([P, D], F32, tag="nat_i")
                tl = d_pool.tile([P, D], F32, tag="nat_t")
                nc.sync.dma_start(il[:], iln3[:, t, :])
                nc.sync.dma_start(tl[:], tln3[:, t, :])
                scr = d_pool.tile([P, D], F32, tag="prodscr")
                nc.vector.tensor_tensor(scr[:], il[:], tl[:], OP.mult)
                nc.vector.reduce_sum(dgt[:, t : t + 1], scr[:], axis=AX)
            nc.sync.dma_start(out_dg, dgt[:])

            # ---- the two logsumexp bands ----
            for side, (lhs_src, rhs_src) in enumerate(((itl, ttf), (ttl, itf))):
                lhsT = lhs_pool.tile([P, KO, NL], BF16, tag="lhst")
                nc.sync.dma_start(lhsT[:], lhs_src.rearrange("(ko p) m -> p ko m", p=P))
                rhs3 = rhs_src.rearrange("(ko p) n -> p ko n", p=P)

                # negated running max, ping-pong on dim 1; running sum
                nmax = st_pool.tile([P, 2, MT], F32, tag=f"nmax{side}")
                runsum = st_pool.tile([P, MT], F32, tag=f"runsum{side}")

                for j in range(NCH):
                    rhs = rhs_pool.tile([P, KO, CH], BF16, tag="rhs")
                    nc.sync.dma_start(rhs[:], rhs3[:, :, j * CH : (j + 1) * CH])
                    for t in range(MT):
                        ps = psum_pool.tile([P, CH], F32, tag="ps")
                        for js in range(NSUB):
                            for k in range(KO):
                                nc.tensor.matmul(
                                    ps[:, js * FREE : (js + 1) * FREE],
                                    lhsT[:, k, t * P : (t + 1) * P],
                                    rhs[:, k, js * FREE : (js + 1) * FREE],
                                    start=(k == 0),
                                    stop=(k == KO - 1),
                                )
                        cur = nmax[:, j % 2, t : t + 1]
                        E = e_pool.tile([P, CH], BF16, tag="exp")
                        if j == 0:
                            nc.vector.reduce_max(cur, ps[:], axis=AX, negate=True)
                            nc.scalar.activation(
                                E[:], ps[:], ACT.Exp, bias=cur, scale=1.0,
                                accum_out=runsum[:, t : t + 1],
                            )
                        else:
                            prev = nmax[:, (j - 1) % 2, t : t + 1]
                            nb = sm_pool.tile([P, 1], F32, tag="nb")
                            nc.vector.reduce_max(nb[:], ps[:], axis=AX, negate=True)
                            nc.vector.tensor_tensor(cur, prev, nb[:], OP.min)
                            f = sm_pool.tile([P, 1], F32, tag="f")
                            # f = exp(oldmax - newmax) = exp(-prev_neg + cur_neg)
                            nc.scalar.activation(
                                f[:], prev, ACT.Exp, bias=cur, scale=-1.0,
                            )
                            bs = sm_pool.tile([P, 1], F32, tag="bs")
                            nc.scalar.activation(
                                E[:], ps[:], ACT.Exp, bias=cur, scale=1.0,
                                accum_out=bs[:],
                            )
                            # runsum = runsum * f + bs
                            nc.vector.scalar_tensor_tensor(
                                runsum[:, t : t + 1], runsum[:, t : t + 1],
                                f[:], bs[:], OP.mult, OP.add,
                            )

                # lse = ln(runsum) + max = ln(runsum) - nmax_final
                lnt = st_pool.tile([P, MT], F32, tag=f"ln{side}")
                nc.scalar.activation(lnt[:], runsum[:], ACT.Ln)
                lse_sb = st_pool.tile([P, MT], F32, tag=f"lse{side}")
                nc.vector.tensor_sub(lse_sb[:], lnt[:], nmax[:, (NCH - 1) % 2, :])
                nc.sync.dma_start(out_lse[side], lse_sb[:])

    nc.compile()
    return nc


_NC = None


def _get_program():
    global _NC
    if _NC is None:
        _NC = _build_program()
    return _NC


def _make_in_maps(image_features, text_features, logit_scale):
    scale = float(np.asarray(logit_scale))
    I = np.asarray(image_features, dtype=np.float32)
    T = np.asarray(text_features, dtype=np.float32)
    Is = I * scale
    ITf = np.ascontiguousarray(Is.T).astype(ml_dtypes.bfloat16)
    TTf = np.ascontiguousarray(T.T).astype(ml_dtypes.bfloat16)
    in_maps = []
    for c in range(C):
        sl = slice(c * NL, (c + 1) * NL)
        in_maps.append({
            "itf": ITf,
            "ttf": TTf,
            "itl": np.ascontiguousarray(ITf[:, sl]),
            "ttl": np.ascontiguousarray(TTf[:, sl]),
            "iln": np.ascontiguousarray(Is[sl]),
            "tln": np.ascontiguousarray(T[sl]),
        })
    return in_maps


def _combine(results):
    """Host-side gather + final scalar reduction in float64."""
    def flat(name):
        # [P, MT] per core, row = t*128 + p  ->  [N]
        return np.concatenate(
            [np.asarray(r[name], dtype=np.float64).T.reshape(-1) for r in results]
        )

    lse_i = flat("lse_i")
    lse_t = flat("lse_t")
    dg = flat("dg")
    loss = 0.5 * (np.mean(lse_i - dg) + np.mean(lse_t - dg))
    return np.asarray(loss, dtype=np.float32)


def kernel(image_features, text_features, logit_scale):
    nc = _get_program()
    in_maps = _make_in_maps(image_features, text_features, logit_scale)
    results = bass2jax.run_bass_via_pjrt(nc, in_maps, n_cores=C)
    return _combine(results)
